# revision 1
# baseline (speedup 1.0000x reference)
"""Trainium2 Bass kernel for CombineAttention (B=2, T=4096, sT=1024, C=1024, H=16, D=64).

Sharding: 8 cores = 2 batches x 4 head-groups (4 heads each).
Host pre-transposes activations/weights so every on-device matmul has its
contraction dim on partitions; the monotonic mask (query i attends keys
<= 4i+3) becomes a block-causal structure handled by suffix-restricted
matmuls plus one small static (128,32) diagonal-band mask.

Precision: fp16 everywhere (full PE rate, FWL weight loads, ~2^-11
element error so quantization noise is ~8x below bf16) except the
attention-weights path: exp(scores) can reach e^40, beyond fp16 range,
so exp and v are bf16 and the attn@v matmul runs in bf16. PSUM
accumulation is fp32 throughout; softmax needs no max-subtraction, and
a ones-column appended to v yields the softmax normalizer for free.

Per-core pipeline:
  qsT = WqT.T @ sxT          (256,1024)   q-scale folded into WqT on host
  kT  = WkT.T @ xT           (256,4096)   x streamed in 512-key slices
  v   = xT.T  @ WvT          (4096,256) + ones column
  per head: scoresT = kT_tile.T @ qsT ; exp ; mask band ; yT_aug = v_aug.T @ expT
  normalize rows by the ones-column sum; out_partial = yT.T_chunks @ WcT
Host sums the 4 head-group partials per batch.
"""

import math
from contextlib import ExitStack

import numpy as np
import ml_dtypes

import concourse.bass as bass
import concourse.tile as tile
from concourse import bacc, mybir
from concourse.bass import ts, ds

BF16 = mybir.dt.bfloat16
FP16 = mybir.dt.float16
FP32 = mybir.dt.float32

B = 2
C = 1024
T = 4096
ST = 1024
H = 16
D = 64
HO = 256          # head-group output channels per core (4 heads)
NCC = C // 128    # 8 contraction chunks
NTT = T // 128    # 32 key tiles
NKC = T // 512    # 8 key slices (projection streaming)
NQC = ST // 512   # 2 query chunks (attention)
NQT = ST // 128   # 8 query tiles (c-projection)
WARM_MMS = 24     # PE warmup burst to lift the HAM clock gate early


def build_nc(masked: bool = True):
    nc = bacc.Bacc("TRN2", target_bir_lowering=False, debug=False, num_devices=8)
    xT = nc.dram_tensor("xT", [C, T], FP16, kind="ExternalInput").ap()
    sxT = nc.dram_tensor("sxT", [C, ST], FP16, kind="ExternalInput").ap()
    wq = nc.dram_tensor("wq", [C, HO], FP16, kind="ExternalInput").ap()
    wk = nc.dram_tensor("wk", [C, HO], FP16, kind="ExternalInput").ap()
    wv = nc.dram_tensor("wv", [C, HO], FP16, kind="ExternalInput").ap()
    wc = nc.dram_tensor("wc", [HO, C], FP16, kind="ExternalInput").ap()
    maskd = nc.dram_tensor("mask", [128, 32], BF16, kind="ExternalInput").ap()
    out = nc.dram_tensor("out", [ST, C], FP32, kind="ExternalOutput").ap()

    with tile.TileContext(nc) as tc, ExitStack() as ctx:
        const = ctx.enter_context(tc.tile_pool(name="const", bufs=1))
        big = ctx.enter_context(tc.tile_pool(name="big", bufs=1))
        xsl_pool = ctx.enter_context(tc.tile_pool(name="xsl", bufs=6))
        work = ctx.enter_context(tc.tile_pool(name="work", bufs=16))
        nrm = ctx.enter_context(tc.tile_pool(name="nrm", bufs=4))
        outw = ctx.enter_context(tc.tile_pool(name="outw", bufs=3))

        wq_sb = const.tile([128, NCC, HO], FP16, tag="wq")
        wk_sb = const.tile([128, NCC, HO], FP16, tag="wk")
        wv_sb = const.tile([128, NCC, HO], FP16, tag="wv")
        wc_sb = const.tile([128, 2, C], FP16, tag="wc")
        mask_sb = const.tile([128, 32], BF16, tag="mask")
        warm_sb = const.tile([128, 512], BF16, tag="warm")

        kT_sb = big.tile([128, 2, T], FP16, tag="kT")
        qsT_sb = big.tile([128, 2, ST], FP16, tag="qsT")
        v_sb = big.tile([128, NTT, 4, 65], BF16, tag="v")
        yT_sb = [
            big.tile([128, 2, 512], FP16, tag=f"yT{qc}", name=f"yT{qc}")
            for qc in range(NQC)
        ]

        nc.vector.memset(warm_sb[:], 0.125)

        with tc.tile_pool(name="psA", bufs=2, space="PSUM") as pp, \
             tc.tile_pool(name="psS", bufs=2, space="PSUM") as scp, \
             tc.tile_pool(name="psV", bufs=2, space="PSUM") as avp:

            # ---- PE warmup: keep the HAM clock gate open through the DMA
            # prologue so the first real matmuls run at 2.4 GHz ----
            wps = pp.tile([128, 512], FP32, tag="proj", name="warmps")
            for i in range(WARM_MMS):
                nc.tensor.matmul(
                    wps[:], warm_sb[:, 0:128], warm_sb[:], start=True, stop=True
                )

            # ---- q projection: qc0 first (everything downstream needs it
            # first), then qc1 ----
            sxsl = []
            for qc in range(NQC):
                sl = xsl_pool.tile([128, NCC, 512], FP16, tag="xsl", name=f"sxsl{qc}")
                sxsl.append(sl)
            for cc in range(NCC):
                nc.sync.dma_start(wq_sb[:, cc, :], wq[ts(cc, 128), :])
                nc.sync.dma_start(sxsl[0][:, cc, :], sxT[ts(cc, 128), ts(0, 512)])
            for cc in range(NCC):
                nc.sync.dma_start(wk_sb[:, cc, :], wk[ts(cc, 128), :])
                nc.sync.dma_start(sxsl[1][:, cc, :], sxT[ts(cc, 128), ts(1, 512)])
            nc.sync.dma_start(wv_sb[:], wv.rearrange("(cc p) o -> p cc o", p=128))
            nc.sync.dma_start(mask_sb[:], maskd[:])

            for qc in range(NQC):
                for ot in range(2):
                    ps = pp.tile([128, 512], FP32, tag="proj", name=f"pq{qc}{ot}")
                    for cc in range(NCC):
                        nc.tensor.matmul(
                            ps[:],
                            wq_sb[:, cc, ts(ot, 128)],
                            sxsl[qc][:, cc, :],
                            start=(cc == 0),
                            stop=(cc == NCC - 1),
                        )
                    nc.vector.tensor_copy(qsT_sb[:, ot, ts(qc, 512)], ps[:])

            def proj_slice_pair(kc0):
                """k/v projections for key slices kc0, kc0+1 (stationary reuse)."""
                xsl = []
                for j in range(2):
                    sl = xsl_pool.tile(
                        [128, NCC, 512], FP16, tag="xsl", name=f"xsl{kc0 + j}"
                    )
                    for cc in range(NCC):
                        nc.sync.dma_start(
                            sl[:, cc, :], xT[ts(cc, 128), ts(kc0 + j, 512)]
                        )
                    xsl.append(sl)
                for ot in range(2):
                    pk = [pp.tile([128, 512], FP32, tag="proj", name=f"pk{j}")
                          for j in range(2)]
                    for cc in range(NCC):
                        for j in range(2):
                            nc.tensor.matmul(
                                pk[j][:],
                                wk_sb[:, cc, ts(ot, 128)],
                                xsl[j][:, cc, :],
                                start=(cc == 0),
                                stop=(cc == NCC - 1),
                            )
                    for j in range(2):
                        nc.vector.tensor_copy(
                            kT_sb[:, ot, ts(kc0 + j, 512)], pk[j][:]
                        )
                for j in range(2):
                    for tl in range(4):
                        tt = 4 * (kc0 + j) + tl
                        ps = pp.tile([128, 512], FP32, tag="proj", name="pv")
                        pv = ps[:, 0:256]
                        for cc in range(NCC):
                            nc.tensor.matmul(
                                pv,
                                xsl[j][:, cc, ts(tl, 128)],
                                wv_sb[:, cc, :],
                                start=(cc == 0),
                                stop=(cc == NCC - 1),
                            )
                        nc.vector.tensor_copy(
                            v_sb[:, tt, :, 0:64], pv.rearrange("p (h d) -> p h d", h=4)
                        )
                        nc.vector.memset(v_sb[:, tt, :, 64:65], 1.0)

            av_tiles = {}
            ex_tiles = {}

            def attn_scores(ot, qc, t0, t1):
                """scoresT + exp + mask for key tiles [t0,t1) of heads
                (2*ot, 2*ot+1), queries [512*qc, 512*qc+512). Emitted ahead
                of the matching attn_av to keep ScalarE fed while the PE is
                busy with projection slices."""
                ntiles = (16 * (qc + 1)) if masked else NTT
                for tt in range(t0, min(t1, ntiles)):
                    r = tt - 16 * qc if masked else -1  # >= 0: diagonal-band tile
                    col0 = 32 * r if r >= 0 else 0
                    width = 512 - col0
                    # both heads' scores go into one 2-bank psum tile, h0 at
                    # the end of bank 0 and h1 at the start of bank 1, so a
                    # single gap-free ACTIVATE (352-cycle fixed cost) covers
                    # the pair
                    base = 512 - width
                    sc = scp.tile([128, 1024], FP32, tag="sc")
                    for h in range(2):
                        row = ds(64 * h, 64)
                        nc.tensor.matmul(
                            sc[:, ds(base + width * h, width)],
                            kT_sb[row, ot, ts(tt, 128)],
                            qsT_sb[row, ot, ds(512 * qc + col0, width)],
                            start=True,
                            stop=True,
                        )
                    ex = work.tile([128, 1024], BF16, tag="exp", name=f"ex{ot}{qc}{tt}")
                    nc.scalar.activation(
                        ex[:, ds(base, 2 * width)],
                        sc[:, ds(base, 2 * width)],
                        mybir.ActivationFunctionType.Exp,
                    )
                    if r >= 0:
                        exb = ex[:, ds(base, 2 * width)].rearrange(
                            "p (g x) -> p g x", g=2
                        )[:, :, 0:32]
                        nc.vector.tensor_mul(
                            exb,
                            exb,
                            mask_sb[:].unsqueeze(1).broadcast_to([128, 2, 32]),
                        )
                    ex_tiles[(ot, qc, tt)] = ex

            def attn_av(ot, qc, t0, t1):
                """attn @ v_aug accumulation for key tiles [t0,t1)."""
                ntiles = (16 * (qc + 1)) if masked else NTT
                if t0 == 0:
                    av_tiles[(ot, qc)] = [
                        avp.tile([65, 512], FP32, tag="av", name=f"av{ot}{qc}{hh}")
                        for hh in range(2)
                    ]
                avps = av_tiles[(ot, qc)]
                for tt in range(t0, min(t1, ntiles)):
                    r = tt - 16 * qc if masked else -1
                    col0 = 32 * r if r >= 0 else 0
                    width = 512 - col0
                    base = 512 - width
                    ex = ex_tiles.pop((ot, qc, tt))
                    for h in range(2):
                        nc.tensor.matmul(
                            avps[h][:, ds(col0, width)],
                            v_sb[:, tt, 2 * ot + h, :],
                            ex[:, ds(base + width * h, width)],
                            start=(tt == 0),
                            stop=(tt == ntiles - 1),
                        )

            def attn_range(ot, qc, t0, t1):
                attn_scores(ot, qc, t0, t1)
                attn_av(ot, qc, t0, t1)

            def attn_norm(ot, qc):
                # normalize: y = yT_unnorm / l  (l = ones-column row of av).
                # Copy av to SBUF first: frees the PSUM slot for the next
                # unit immediately, and custom-DVE recip cannot read PSUM.
                avps = av_tiles.pop((ot, qc))
                for h in range(2):
                    lsb = nrm.tile([1, 512], FP32, tag="lsb")
                    nc.vector.tensor_copy(lsb[:], avps[h][64:65, :])
                    linv = nrm.tile([1, 512], FP32, tag="linv")
                    nc.vector.reciprocal_approx_fast(linv[:], lsb[:])
                    avcp = nrm.tile([64, 512], FP32, tag="avcp")
                    nc.vector.tensor_copy(avcp[:], avps[h][0:64, :])
                    bc = nrm.tile([64, 512], FP32, tag="bc")
                    # gpsimd queue: keeps this dependent DMA out of the sync
                    # queue, whose in-order issue would stall later x slices
                    nc.gpsimd.dma_start(
                        bc[:], linv[:].unsqueeze(1).broadcast_to([1, 64, 512])
                    )
                    nc.vector.tensor_mul(
                        yT_sb[qc][ds(64 * h, 64), ot, :],
                        avcp[:],
                        bc[:],
                    )

            def cproj():
                for nt in range(NQT):
                    po = [pp.tile([128, 512], FP32, tag="proj", name=f"po{ec}")
                          for ec in range(2)]
                    for kk in range(2):
                        for ec in range(2):
                            nc.tensor.matmul(
                                po[ec][:],
                                yT_sb[nt // 4][:, kk, ts(nt % 4, 128)],
                                wc_sb[:, kk, ts(ec, 512)],
                                start=(kk == 0),
                                stop=(kk == 1),
                            )
                    for ec in range(2):
                        osb = outw.tile([128, 512], FP32, tag="osb")
                        nc.vector.tensor_copy(osb[:], po[ec][:])
                        nc.sync.dma_start(out[ts(nt, 128), ts(ec, 512)], osb[:])

            if masked:
                proj_slice_pair(0)
                attn_range(0, 0, 0, 8)
                attn_scores(1, 0, 0, 8)
                proj_slice_pair(2)
                attn_range(0, 0, 8, 16)
                attn_norm(0, 0)
                attn_av(1, 0, 0, 8)
                attn_range(1, 0, 8, 16)
                attn_norm(1, 0)
                attn_scores(0, 1, 0, 16)
                proj_slice_pair(4)
                attn_av(0, 1, 0, 16)
                attn_scores(1, 1, 0, 8)
                proj_slice_pair(6)
                for kk in range(2):
                    nc.sync.dma_start(wc_sb[:, kk, :], wc[ts(kk, 128), :])
                attn_range(0, 1, 16, 32)
                attn_norm(0, 1)
                attn_av(1, 1, 0, 8)
                attn_range(1, 1, 8, 32)
                attn_norm(1, 1)
            else:
                for kc in range(0, NKC, 2):
                    proj_slice_pair(kc)
                for kk in range(2):
                    nc.sync.dma_start(wc_sb[:, kk, :], wc[ts(kk, 128), :])
                for qc in range(NQC):
                    for ot in range(2):
                        attn_range(ot, qc, 0, NTT)
                        attn_norm(ot, qc)
            cproj()
            # tail warmers: keep the PE clock gate open through the final
            # norm chains so the c-projection runs at full clock
            wps2 = scp.tile([128, 512], FP32, tag="sc", name="warmps2")
            for i in range(16):
                nc.tensor.matmul(
                    wps2[:], warm_sb[:, 0:128], warm_sb[:], start=True, stop=True
                )

    nc.compile()
    return nc


_NC_CACHE = {}


def _get_nc(masked: bool):
    if masked not in _NC_CACHE:
        _NC_CACHE[masked] = build_nc(masked)
    return _NC_CACHE[masked]


def _shard_inputs(x, sx, Wq, Wk, Wv, Wc, qm):
    f16 = np.float16
    bf = ml_dtypes.bfloat16
    t_len = x.shape[1]
    qscale = math.log(t_len) / math.sqrt(D)
    qmfull = np.tile(np.asarray(qm, np.float32), 4) * qscale  # (256,)

    tk = np.arange(128)[:, None]
    cl = np.arange(32)[None, :]
    mask = (cl >= tk // 4).astype(np.float32).astype(bf)

    in_maps = []
    for b in range(B):
        xT = np.ascontiguousarray(x[b].T).astype(f16)
        sxT = np.ascontiguousarray(sx[b].T).astype(f16)
        for hg in range(4):
            sl = slice(hg * HO, (hg + 1) * HO)
            in_maps.append(
                {
                    "xT": xT,
                    "sxT": sxT,
                    "wq": np.ascontiguousarray(
                        (Wq[sl, :] * qmfull[:, None]).T
                    ).astype(f16),
                    "wk": np.ascontiguousarray(Wk[sl, :].T).astype(f16),
                    "wv": np.ascontiguousarray(Wv[sl, :].T).astype(f16),
                    "wc": np.ascontiguousarray(Wc[:, sl].T).astype(f16),
                    "mask": mask,
                }
            )
    return in_maps


def _run(inputs, trace=False):
    from concourse.bass_utils import run_bass_kernel_spmd

    x = np.asarray(inputs["x"], np.float32)
    sx = np.asarray(inputs["sx"], np.float32)
    Wq = np.asarray(inputs["Wq"], np.float32)
    Wk = np.asarray(inputs["Wk"], np.float32)
    Wv = np.asarray(inputs["Wv"], np.float32)
    Wc = np.asarray(inputs["Wc"], np.float32)
    qm = np.asarray(inputs["qm"], np.float32)
    causal = int(np.asarray(inputs.get("causal", 1)))
    masked = bool(causal) and sx.shape[1] != x.shape[1]

    nc = _get_nc(masked)
    in_maps = _shard_inputs(x, sx, Wq, Wk, Wv, Wc, qm)
    kwargs = {}
    if trace:
        kwargs = dict(trace=True, trace_cores=list(range(8)))
    res = run_bass_kernel_spmd(nc, in_maps, core_ids=list(range(8)), **kwargs)

    out = np.zeros((B, ST, C), np.float32)
    for b in range(B):
        for hg in range(4):
            out[b] += res.results[b * 4 + hg]["out"]
    return out, res


def kernel(**inputs):
    out, _ = _run(inputs, trace=False)
    return out


def kernel_traced(**inputs):
    out, res = _run(inputs, trace=True)
    return out, res



# revision 8
# speedup vs baseline: 1.0934x; 1.0934x over previous
"""Trainium2 Bass kernel for CombineAttention (B=2, T=4096, sT=1024, C=1024, H=16, D=64).

Sharding: 8 cores = 2 batches x 4 head-groups (4 heads each).
Host pre-transposes activations/weights so every on-device matmul has its
contraction dim on partitions; the monotonic mask (query i attends keys
<= 4i+3) becomes a block-causal structure handled by suffix-restricted
matmuls plus one small static (128,32) diagonal-band mask.

Precision: fp16 everywhere (full PE rate, FWL weight loads) except the
attention-weights path: exp(scores) can reach e^40, beyond fp16 range,
so exp and v are bf16 and the attn@v matmul runs in bf16. PSUM
accumulation is fp32; softmax needs no max-subtraction, and a
ones-column appended to v yields the softmax normalizer for free.
Output partials are stored fp16 and summed on host in fp32.

v2 schedule: single fine-grained instruction stream that keeps the PE
warm (HAM clock gate) and the ScalarE exp pipe full:
  - q/k/v projections chopped into ~1-3.5us quanta, interleaved between
    attention score/av tile pairs so the PE never waits on ScalarE;
  - scores for the last unit's full tiles are emitted early (deep ex
    buffer) because that unit has no projection work left to hide its
    exp latency;
  - softmax normalization broadcasts 1/l via a K=1 PE matmul instead of
    a DMA (keeps the tail chain ~2us instead of ~11us);
  - c-projection for query chunk 0 runs mid-kernel; out stores are fp16
    on the gpsimd DMA queue so they never block x-slice loads on the
    in-order sync queue.
"""

import math
from contextlib import ExitStack

import numpy as np
import ml_dtypes

import concourse.bass as bass
import concourse.tile as tile
from concourse import bacc, mybir
from concourse.bass import ts, ds

BF16 = mybir.dt.bfloat16
FP16 = mybir.dt.float16
FP32 = mybir.dt.float32

B = 2
C = 1024
T = 4096
ST = 1024
H = 16
D = 64
HO = 256          # head-group output channels per core (4 heads)
NCC = C // 128    # 8 contraction chunks
NTT = T // 128    # 32 key tiles
NKC = T // 512    # 8 key slices (projection streaming)
NQC = ST // 512   # 2 query chunks (attention)
NQT = ST // 128   # 8 query tiles (c-projection)
WARM_MMS = 8      # PE warmup burst; real q/k work continues the warming


def build_nc(masked: bool = True):
    nc = bacc.Bacc("TRN2", target_bir_lowering=False, debug=False, num_devices=8)
    xT = nc.dram_tensor("xT", [C, T], FP16, kind="ExternalInput").ap()
    sxT = nc.dram_tensor("sxT", [C, ST], FP16, kind="ExternalInput").ap()
    wq = nc.dram_tensor("wq", [C, HO], FP16, kind="ExternalInput").ap()
    wk = nc.dram_tensor("wk", [C, HO], FP16, kind="ExternalInput").ap()
    wv = nc.dram_tensor("wv", [C, HO], FP16, kind="ExternalInput").ap()
    wc = nc.dram_tensor("wc", [HO, C], FP16, kind="ExternalInput").ap()
    maskd = nc.dram_tensor("mask", [128, 32], BF16, kind="ExternalInput").ap()
    out = nc.dram_tensor("out", [ST, C], FP16, kind="ExternalOutput").ap()

    with tile.TileContext(nc) as tc, ExitStack() as ctx:
        const = ctx.enter_context(tc.tile_pool(name="const", bufs=1))
        big = ctx.enter_context(tc.tile_pool(name="big", bufs=1))
        xsl_pool = ctx.enter_context(tc.tile_pool(name="xsl", bufs=6))
        work = ctx.enter_context(tc.tile_pool(name="work", bufs=20))
        nrm = ctx.enter_context(tc.tile_pool(name="nrm", bufs=4))
        outw = ctx.enter_context(tc.tile_pool(name="outw", bufs=3))

        wq_sb = const.tile([128, NCC, HO], FP16, tag="wq")
        wk_sb = const.tile([128, NCC, HO], FP16, tag="wk")
        wv_sb = const.tile([128, NCC, HO], FP16, tag="wv")
        wc_sb = const.tile([128, 2, C], FP16, tag="wc")
        mask_sb = const.tile([128, 32], BF16, tag="mask")
        warm_sb = const.tile([128, 512], BF16, tag="warm")
        ones_sb = const.tile([1, 64], BF16, tag="ones")

        kT_sb = big.tile([128, 2, T], FP16, tag="kT")
        qsT_sb = big.tile([128, 2, ST], FP16, tag="qsT")
        v_sb = big.tile([128, NTT, 4, 65], BF16, tag="v")
        yT_sb = [
            big.tile([128, 2, 512], FP16, tag=f"yT{qc}", name=f"yT{qc}")
            for qc in range(NQC)
        ]

        nc.gpsimd.memset(warm_sb[:], 0.125)
        nc.gpsimd.memset(ones_sb[:], 1.0)

        with tc.tile_pool(name="psA", bufs=2, space="PSUM") as pp, \
             tc.tile_pool(name="psS", bufs=2, space="PSUM") as scp, \
             tc.tile_pool(name="psV", bufs=2, space="PSUM") as avp:

            # ---- PE warmup: bridge from kernel start until the first
            # x/weight slices land; real projections continue the burst ----
            wps = pp.tile([128, 512], FP32, tag="proj", name="warmps")
            for i in range(WARM_MMS):
                nc.tensor.matmul(
                    wps[:], warm_sb[:, 0:128], warm_sb[:], start=True, stop=True
                )

            # ---------------- DMA emission helpers ----------------
            def dma_pair_x(kc0):
                """Start DMAs for key slices kc0, kc0+1; returns xsl tiles."""
                xsl = []
                for j in range(2):
                    sl = xsl_pool.tile(
                        [128, NCC, 512], FP16, tag="xsl", name=f"xsl{kc0 + j}"
                    )
                    for cc in range(NCC):
                        nc.sync.dma_start(
                            sl[:, cc, :], xT[ts(cc, 128), ts(kc0 + j, 512)]
                        )
                    xsl.append(sl)
                return xsl

            # ---------------- PE work quanta ----------------
            def qp_quantum(sxsl, qc, ot):
                """q projection for (query chunk qc, channel half ot): 8 MMs."""
                ps = pp.tile([128, 512], FP32, tag="proj", name=f"pq{qc}{ot}")
                for cc in range(NCC):
                    nc.tensor.matmul(
                        ps[:],
                        wq_sb[:, cc, ts(ot, 128)],
                        sxsl[qc][:, cc, :],
                        start=(cc == 0),
                        stop=(cc == NCC - 1),
                    )
                nc.vector.tensor_copy(qsT_sb[:, ot, ts(qc, 512)], ps[:])

            def k_quantum(xsl, kc0, ot):
                """k projection for slices kc0,kc0+1 (one channel half): 16 MMs
                sharing stationary loads across the j-pair."""
                pk = [pp.tile([128, 512], FP32, tag="proj", name=f"pk{j}")
                      for j in range(2)]
                for cc in range(NCC):
                    for j in range(2):
                        nc.tensor.matmul(
                            pk[j][:],
                            wk_sb[:, cc, ts(ot, 128)],
                            xsl[j][:, cc, :],
                            start=(cc == 0),
                            stop=(cc == NCC - 1),
                        )
                for j in range(2):
                    nc.vector.tensor_copy(kT_sb[:, ot, ts(kc0 + j, 512)], pk[j][:])

            def v_quantum(xsl, kc0, j, tl):
                """v projection for one 128-key tile: 8 MMs of N=256."""
                tt = 4 * (kc0 + j) + tl
                ps = pp.tile([128, 512], FP32, tag="proj", name="pv")
                pv = ps[:, 0:256]
                for cc in range(NCC):
                    nc.tensor.matmul(
                        pv,
                        xsl[j][:, cc, ts(tl, 128)],
                        wv_sb[:, cc, :],
                        start=(cc == 0),
                        stop=(cc == NCC - 1),
                    )
                nc.vector.tensor_copy(
                    v_sb[:, tt, :, 0:64], pv.rearrange("p (h d) -> p h d", h=4)
                )
                nc.vector.memset(v_sb[:, tt, :, 64:65], 1.0)

            ex_tiles = {}
            av_tiles = {}

            def tile_geom(qc, tt):
                r = tt - 16 * qc if masked else -1  # >= 0: diagonal-band tile
                col0 = 32 * r if r >= 0 else 0
                width = 512 - col0
                base = 512 - width
                return r, col0, width, base

            def ntiles_of(qc):
                return (16 * (qc + 1)) if masked else NTT

            def S(ot, qc, tt):
                """scoresT + exp + band-mask for one 128-key tile of heads
                (2*ot, 2*ot+1), queries [512*qc, 512*qc+512)."""
                r, col0, width, base = tile_geom(qc, tt)
                # both heads' scores go into one 2-bank psum tile, h0 at the
                # end of bank 0 and h1 at the start of bank 1, so a single
                # gap-free ACTIVATE (352-cycle fixed cost) covers the pair;
                # the two heads' matmuls run concurrently on the upper/lower
                # halves of the PE array (row tiling via base_partition)
                sc = scp.tile([128, 1024], FP32, tag="sc")
                for h in range(2):
                    row = ds(64 * h, 64)
                    nc.tensor.matmul(
                        sc[:, ds(base + width * h, width)],
                        kT_sb[row, ot, ts(tt, 128)],
                        qsT_sb[row, ot, ds(512 * qc + col0, width)],
                        start=True,
                        stop=True,
                    )
                ex = work.tile([128, 1024], BF16, tag="exp", name=f"ex{ot}{qc}{tt}")
                nc.scalar.activation(
                    ex[:, ds(base, 2 * width)],
                    sc[:, ds(base, 2 * width)],
                    mybir.ActivationFunctionType.Exp,
                )
                if r >= 0:
                    exb = ex[:, ds(base, 2 * width)].rearrange(
                        "p (g x) -> p g x", g=2
                    )[:, :, 0:32]
                    nc.vector.tensor_mul(
                        exb,
                        exb,
                        mask_sb[:].unsqueeze(1).broadcast_to([128, 2, 32]),
                    )
                ex_tiles[(ot, qc, tt)] = ex

            def A(ot, qc, tt):
                """attn @ v_aug accumulation for one key tile."""
                ntiles = ntiles_of(qc)
                if tt == 0:
                    av_tiles[(ot, qc)] = [
                        avp.tile([65, 512], FP32, tag="av", name=f"av{ot}{qc}{hh}")
                        for hh in range(2)
                    ]
                avps = av_tiles[(ot, qc)]
                r, col0, width, base = tile_geom(qc, tt)
                ex = ex_tiles.pop((ot, qc, tt))
                for h in range(2):
                    nc.tensor.matmul(
                        avps[h][:, ds(col0, width)],
                        v_sb[:, tt, 2 * ot + h, :],
                        ex[:, ds(base + width * h, width)],
                        start=(tt == 0),
                        stop=(tt == ntiles - 1),
                    )

            norm_state = {}

            def norm_dve(ot, qc):
                """First half of y = yT_unnorm / l: copy av+l to SBUF and
                compute 1/l (all DVE). Emitted right after the unit's last av
                matmul so the chain runs while the PE does other work."""
                avps = av_tiles.pop((ot, qc))
                st = []
                for h in range(2):
                    lsb = nrm.tile([1, 512], FP32, tag="lsb")
                    nc.vector.tensor_copy(lsb[:], avps[h][64:65, :])
                    avsb = nrm.tile([64, 512], FP32, tag="avsb", name=f"avsb{h}")
                    nc.vector.tensor_copy(avsb[:], avps[h][0:64, :])
                    linv = nrm.tile([1, 512], FP32, tag="linv")
                    # custom-DVE recip needs a partition-0 SBUF input
                    nc.vector.reciprocal_approx_fast(linv[:], lsb[:])
                    linvb = nrm.tile([1, 512], BF16, tag="linvb")
                    nc.vector.tensor_copy(linvb[:], linv[:])
                    st.append((avsb, linvb))
                norm_state[(ot, qc)] = st

            def norm_fin(ot, qc):
                """Second half: K=1 PE matmul broadcasts 1/l across 64
                partitions, one DVE multiply writes normalized yT (fp16).
                Must precede the next unit's first av matmul (psum reuse)."""
                st = norm_state.pop((ot, qc))
                bcs = []
                for h, (avsb, linvb) in enumerate(st):
                    bc = avp.tile([64, 512], FP32, tag="av", name=f"bc{h}")
                    nc.tensor.matmul(
                        bc[:], ones_sb[:], linvb[:], start=True, stop=True
                    )
                    bcs.append(bc)
                for h, (avsb, linvb) in enumerate(st):
                    nc.vector.tensor_mul(
                        yT_sb[qc][ds(64 * h, 64), ot, :], avsb[:], bcs[h][:]
                    )

            def cp(nt):
                """c-projection + fp16 store for one 128-query tile."""
                po = [pp.tile([128, 512], FP32, tag="proj", name=f"po{ec}")
                      for ec in range(2)]
                for kk in range(2):
                    for ec in range(2):
                        nc.tensor.matmul(
                            po[ec][:],
                            yT_sb[nt // 4][:, kk, ts(nt % 4, 128)],
                            wc_sb[:, kk, ts(ec, 512)],
                            start=(kk == 0),
                            stop=(kk == 1),
                        )
                for ec in range(2):
                    osb = outw.tile([128, 512], FP16, tag="osb")
                    nc.vector.tensor_copy(osb[:], po[ec][:])
                    # gpsimd DMA queue: out stores must not block x-slice
                    # loads on the in-order sync queue
                    nc.gpsimd.dma_start(out[ts(nt, 128), ts(ec, 512)], osb[:])

            # ---------------- masked (monotonic) schedule ----------------
            if masked:
                # sxsl tiles for q projection
                sxsl = [
                    xsl_pool.tile([128, NCC, 512], FP16, tag="xsl", name=f"sxsl{qc}")
                    for qc in range(NQC)
                ]
                # DMA priority order: wk + pair0 feed the long k/v pole;
                # wq + sx feed q projection; later pairs stream behind.
                for cc in range(NCC):
                    nc.sync.dma_start(wk_sb[:, cc, :], wk[ts(cc, 128), :])
                xp0 = dma_pair_x(0)
                for cc in range(NCC):
                    nc.sync.dma_start(wq_sb[:, cc, :], wq[ts(cc, 128), :])
                    nc.sync.dma_start(sxsl[0][:, cc, :], sxT[ts(cc, 128), ts(0, 512)])
                for cc in range(NCC):
                    nc.sync.dma_start(sxsl[1][:, cc, :], sxT[ts(cc, 128), ts(1, 512)])
                nc.sync.dma_start(wv_sb[:], wv.rearrange("(cc p) o -> p cc o", p=128))
                nc.sync.dma_start(mask_sb[:], maskd[:])
                xp2 = dma_pair_x(2)
                for kk in range(2):
                    nc.sync.dma_start(wc_sb[:, kk, :], wc[ts(kk, 128), :])
                # pair4 reuses sxsl0's buffer (waits on q-proj qc0), pair6
                # reuses pair0's (waits on k/v of pair0) — emitted now, the
                # semaphores resolve the timing.
                xp4 = dma_pair_x(4)
                xp6 = dma_pair_x(6)

                # Filler queue: projection/cproj quanta consumed between
                # attention tile pairs. Order respects data availability.
                filler = []
                filler.append(lambda: k_quantum(xp0, 0, 0))
                filler.append(lambda: k_quantum(xp0, 0, 1))
                filler.append(lambda: qp_quantum(sxsl, 0, 0))
                filler.append(lambda: qp_quantum(sxsl, 0, 1))
                for j in range(2):
                    for tl in range(4):
                        filler.append(
                            lambda j=j, tl=tl: v_quantum(xp0, 0, j, tl))
                filler.append(lambda: qp_quantum(sxsl, 1, 0))
                filler.append(lambda: qp_quantum(sxsl, 1, 1))
                filler.append(lambda: k_quantum(xp2, 2, 0))
                filler.append(lambda: k_quantum(xp2, 2, 1))
                for j in range(2):
                    for tl in range(4):
                        filler.append(
                            lambda j=j, tl=tl: v_quantum(xp2, 2, j, tl))
                filler.append(lambda: k_quantum(xp4, 4, 0))
                filler.append(lambda: k_quantum(xp4, 4, 1))
                for j in range(2):
                    for tl in range(4):
                        filler.append(
                            lambda j=j, tl=tl: v_quantum(xp4, 4, j, tl))
                filler.append(lambda: k_quantum(xp6, 6, 0))
                filler.append(lambda: k_quantum(xp6, 6, 1))
                for j in range(2):
                    for tl in range(4):
                        filler.append(
                            lambda j=j, tl=tl: v_quantum(xp6, 6, j, tl))
                for nt in range(2):
                    filler.append(lambda nt=nt: cp(nt))

                fill_pos = 0

                def pull(n):
                    nonlocal fill_pos
                    for _ in range(n):
                        if fill_pos < len(filler):
                            filler[fill_pos]()
                            fill_pos += 1

                def pull_through(idx):
                    nonlocal fill_pos
                    while fill_pos <= idx:
                        filler[fill_pos]()
                        fill_pos += 1

                # filler indices: 0-1 k0, 2-3 qp(qc0), 4-11 v(pair0),
                # 12-13 qp(qc1), 14-15 k2, 16-23 v2, 24-25 k4, 26-33 v4,
                # 34-35 k6, 36-43 v6, 44-45 cp(0..1)
                IDX_QP0 = 3
                IDX_K2 = 15
                IDX_K4 = 25
                IDX_K6 = 35

                # --- Unit 0 = (ot 0, qc 0): 16 diagonal tiles ---
                pull_through(IDX_QP0)          # k0 + q(qc0)
                for t in range(0, 8):
                    S(0, 0, t)
                    if t >= 2:
                        A(0, 0, t - 2)
                    pull(1)                    # v(pair0), q(qc1) stream in
                pull_through(IDX_K2)
                for t in range(8, 16):
                    S(0, 0, t)
                    A(0, 0, t - 2)
                    pull(1)
                A(0, 0, 14)
                A(0, 0, 15)
                norm_dve(0, 0)
                # --- Unit 1 = (1, 0): next unit's scores + filler cover the
                # norm chain and av-psum handover ---
                S(1, 0, 0)
                S(1, 0, 1)
                norm_fin(0, 0)
                pull(2)                        # k4 quanta
                for t in range(2, 16):
                    S(1, 0, t)
                    A(1, 0, t - 2)
                    if t % 2 == 0:
                        pull(1)
                A(1, 0, 14)
                A(1, 0, 15)
                norm_dve(1, 0)
                pull_through(IDX_K4)
                # --- Unit 2 = (0, 1): 32 tiles; also pre-emit the first 12 of
                # unit 3's full-tile scores so its exp runs ahead on ScalarE ---
                S(0, 1, 0)
                S(0, 1, 1)
                pull(1)
                norm_fin(1, 0)
                u3_t = 0
                for t in range(2, 32):
                    S(0, 1, t)
                    A(0, 1, t - 2)
                    if t % 2 == 0:
                        pull(1)
                    if t % 2 == 1 and u3_t < 12:
                        S(1, 1, u3_t)
                        u3_t += 1
                    if t == 23:
                        pull_through(IDX_K6)
                A(0, 1, 30)
                A(0, 1, 31)
                norm_dve(0, 1)
                # --- Unit 3 = (1, 1): finish its full-tile scores, then
                # interleave full-tile av with diagonal scores+av so the PE
                # tracks ScalarE's exp progress; cp(2..3) cover the norm ---
                cp(2)
                cp(3)
                norm_fin(0, 1)
                while u3_t < 16:
                    S(1, 1, u3_t)
                    u3_t += 1
                S(1, 1, 16)
                S(1, 1, 17)
                for i in range(16):
                    A(1, 1, i)            # full tile (start group at i == 0)
                    if i >= 3:
                        A(1, 1, 13 + i)   # diagonal tiles 16..28
                    if 18 + i < 32:
                        S(1, 1, 18 + i)
                for t in (29, 30, 31):
                    A(1, 1, t)            # stop group fires on tile 31
                norm_dve(1, 1)
                norm_fin(1, 1)
                for nt in range(4, NQT):
                    cp(nt)
            else:
                # unmasked fallback: coarse sequential schedule
                sxsl = [
                    xsl_pool.tile([128, NCC, 512], FP16, tag="xsl", name=f"sxsl{qc}")
                    for qc in range(NQC)
                ]
                for cc in range(NCC):
                    nc.sync.dma_start(wq_sb[:, cc, :], wq[ts(cc, 128), :])
                    nc.sync.dma_start(sxsl[0][:, cc, :], sxT[ts(cc, 128), ts(0, 512)])
                for cc in range(NCC):
                    nc.sync.dma_start(wk_sb[:, cc, :], wk[ts(cc, 128), :])
                    nc.sync.dma_start(sxsl[1][:, cc, :], sxT[ts(cc, 128), ts(1, 512)])
                nc.sync.dma_start(wv_sb[:], wv.rearrange("(cc p) o -> p cc o", p=128))
                nc.sync.dma_start(mask_sb[:], maskd[:])
                for kk in range(2):
                    nc.sync.dma_start(wc_sb[:, kk, :], wc[ts(kk, 128), :])
                for qc in range(NQC):
                    for ot in range(2):
                        qp_quantum(sxsl, qc, ot)
                for kc in range(0, NKC, 2):
                    xp = dma_pair_x(kc)
                    for ot in range(2):
                        k_quantum(xp, kc, ot)
                    for j in range(2):
                        for tl in range(4):
                            v_quantum(xp, kc, j, tl)
                for qc in range(NQC):
                    for ot in range(2):
                        for t in range(NTT):
                            S(ot, qc, t)
                            if t >= 2:
                                A(ot, qc, t - 2)
                        A(ot, qc, NTT - 2)
                        A(ot, qc, NTT - 1)
                        norm_dve(ot, qc)
                        norm_fin(ot, qc)
                for nt in range(NQT):
                    cp(nt)

    nc.compile()
    return nc


_NC_CACHE = {}


def _get_nc(masked: bool):
    if masked not in _NC_CACHE:
        _NC_CACHE[masked] = build_nc(masked)
    return _NC_CACHE[masked]


def _shard_inputs(x, sx, Wq, Wk, Wv, Wc, qm):
    f16 = np.float16
    bf = ml_dtypes.bfloat16
    t_len = x.shape[1]
    qscale = math.log(t_len) / math.sqrt(D)
    qmfull = np.tile(np.asarray(qm, np.float32), 4) * qscale  # (256,)

    tk = np.arange(128)[:, None]
    cl = np.arange(32)[None, :]
    mask = (cl >= tk // 4).astype(np.float32).astype(bf)

    in_maps = []
    for b in range(B):
        xT = np.ascontiguousarray(x[b].T).astype(f16)
        sxT = np.ascontiguousarray(sx[b].T).astype(f16)
        for hg in range(4):
            sl = slice(hg * HO, (hg + 1) * HO)
            in_maps.append(
                {
                    "xT": xT,
                    "sxT": sxT,
                    "wq": np.ascontiguousarray(
                        (Wq[sl, :] * qmfull[:, None]).T
                    ).astype(f16),
                    "wk": np.ascontiguousarray(Wk[sl, :].T).astype(f16),
                    "wv": np.ascontiguousarray(Wv[sl, :].T).astype(f16),
                    "wc": np.ascontiguousarray(Wc[:, sl].T).astype(f16),
                    "mask": mask,
                }
            )
    return in_maps


def _run(inputs, trace=False):
    from concourse.bass_utils import run_bass_kernel_spmd

    x = np.asarray(inputs["x"], np.float32)
    sx = np.asarray(inputs["sx"], np.float32)
    Wq = np.asarray(inputs["Wq"], np.float32)
    Wk = np.asarray(inputs["Wk"], np.float32)
    Wv = np.asarray(inputs["Wv"], np.float32)
    Wc = np.asarray(inputs["Wc"], np.float32)
    qm = np.asarray(inputs["qm"], np.float32)
    causal = int(np.asarray(inputs.get("causal", 1)))
    masked = bool(causal) and sx.shape[1] != x.shape[1]

    nc = _get_nc(masked)
    in_maps = _shard_inputs(x, sx, Wq, Wk, Wv, Wc, qm)
    kwargs = {}
    if trace:
        kwargs = dict(trace=True, trace_cores=list(range(8)))
    res = run_bass_kernel_spmd(nc, in_maps, core_ids=list(range(8)), **kwargs)

    out = np.zeros((B, ST, C), np.float32)
    for b in range(B):
        for hg in range(4):
            out[b] += np.asarray(res.results[b * 4 + hg]["out"], np.float32)
    return out, res


def kernel(**inputs):
    out, _ = _run(inputs, trace=False)
    return out


def kernel_traced(**inputs):
    out, res = _run(inputs, trace=True)
    return out, res
